# revision 1
# baseline (speedup 1.0000x reference)
"""Sequence-parallel DeepSpeed encoder-decoder block on 8 TRN2 NeuronCores.

Each core owns a 512-token block (cores 0-3 -> sequence 0, cores 4-7 ->
sequence 1) and computes the FULL layer for its tokens: LN1 + QKV (all 16
heads), attention against the whole sequence, attn_ow, residual, LN2, and
the complete MLP. The only communication is an AllGather of k and v within
each sequence's 4-core group; the final output is token-sharded so the host
just concatenates - no ReduceScatter at all.

Weights are replicated (streamed from HBM, pre-tiled host-side for
contiguous DMA). LayerNorms fold gamma into the following weights
host-side; mean subtraction is a rank-1 correction matmul, 1/std a
broadcast multiply at PSUM drain. Softmax runs in transposed score layout
(keys on partitions) with the denominator from a ones-column augmentation
of V^T. Score matmuls for even/odd heads are interleaved so the K=64
contractions pack into the PE array's two row-halves (tile_position).

SBUF is phase-managed with strictly nested pool lifetimes:
global [ P1(qkv weights) ] [ RES(ow/x/resid) [ ATTN ] P3 [ MLP ] ].
"""
from contextlib import ExitStack

import numpy as np
import ml_dtypes

import concourse.bacc as bacc
import concourse.mybir as mybir
import concourse.tile as tile
from concourse import masks
from concourse.bass_utils import run_bass_kernel_spmd

f32 = mybir.dt.float32
bf16 = mybir.dt.bfloat16
f8 = mybir.dt.float8e4
AF = mybir.ActivationFunctionType
ALU = mybir.AluOpType

NC = 8
B, S, D, I = 2, 2048, 1024, 4096
H, HD = 16, 64
T = B * S
TOK = T // NC        # 512 tokens per core
DC = D // 128        # 8 feature chunks
IC = I // 128        # 32 intermediate chunks
NKC = S // 128       # 16 key chunks per sequence
NR = 4               # ranks per sequence group
EPS = 1e-12

_BF = ml_dtypes.bfloat16


def _bf16(a):
    return np.ascontiguousarray(a.astype(_BF))


_F8 = ml_dtypes.float8_e4m3


def _f8(a):
    return np.ascontiguousarray(a.astype(_F8))


def _build():
    nc = bacc.Bacc("TRN2", target_bir_lowering=False, debug=False, num_devices=NC)

    inp = {}
    def din(name, shape, dt):
        inp[name] = nc.dram_tensor(name, shape, dt, kind="ExternalInput")
        return inp[name]

    xbf = din("xbf", [D, TOK], bf16)
    xf = din("xf", [D, TOK], f32)
    wqblk = din("wqblk", [3, DC, 128, DC, 128], f8)
    ncsq = din("ncsq", [1, 3 * D], bf16)
    owblk = din("owblk", [DC, 128, DC, 128], f8)
    w1blk = din("w1blk", [IC, 128, DC, 128], f8)
    ncs1 = din("ncs1", [1, I], bf16)
    w2blk = din("w2blk", [IC, 128, DC, 128], f8)
    outwblk = din("outwblk", [DC, 128, IC, 128], f8)

    outT = nc.dram_tensor("outT", [D, TOK], f32, kind="ExternalOutput")

    RG2 = [[0, 1, 2, 3], [4, 5, 6, 7]]

    with tile.TileContext(nc) as tc:
        with ExitStack() as ctx:
            ep = ctx.enter_context
            # ---- global pools (whole-kernel lifetime) ----
            cons = ep(tc.tile_pool(name="cons", bufs=1))
            qp = ep(tc.tile_pool(name="qp", bufs=1))
            ctxp = ep(tc.tile_pool(name="ctxp", bufs=1))
            sqp = ep(tc.tile_pool(name="sqp", bufs=3))
            rowp = ep(tc.tile_pool(name="rowp", bufs=8))
            rstdp = ep(tc.tile_pool(name="rstdp", bufs=2))
            wfp = ep(tc.tile_pool(name="wfp", bufs=6))
            psp = ep(tc.tile_pool(name="psp", bufs=4, space="PSUM"))
            dram = ep(tc.tile_pool(name="dram", bufs=1, space="DRAM"))

            # ---- constants ----
            ident = cons.tile([128, 128], bf16)
            masks.make_identity(nc, ident[:])
            ones_col = cons.tile([128, 1], bf16)
            nc.gpsimd.memset(ones_col[:], 1.0)
            ones_all = cons.tile([128, 64], bf16)
            nc.gpsimd.memset(ones_all[:], 1.0)
            invD_f = cons.tile([1, 128], f32)
            nc.gpsimd.memset(invD_f[:], 1.0 / D)
            invD_row = cons.tile([1, 128], bf16)
            nc.vector.tensor_copy(invD_row[:], invD_f[:])
            eps_col = cons.tile([128, 1], f32)
            nc.gpsimd.memset(eps_col[:], EPS)
            eps_col2 = cons.tile([128, 1], f32)
            nc.gpsimd.memset(eps_col2[:], 1024.0 * EPS)

            ncsq_row = cons.tile([1, 3 * D], bf16)
            nc.sync.dma_start(ncsq_row[:], ncsq[:])
            ncs1_row = cons.tile([1, I], bf16)
            nc.sync.dma_start(ncs1_row[:], ncs1[:])

            # DRAM scratch for the kv AllGathers
            k_own = dram.tile([D, TOK], f8, tag="k_own", name="k_own")
            v_own = dram.tile([D, TOK], f8, tag="v_own", name="v_own")
            k_ag = dram.tile([NR * D, TOK], f8, tag="k_ag", name="k_ag")
            v_ag = dram.tile([NR * D, TOK], f8, tag="v_ag", name="v_ag")

            def ln_stats(feed_tile_fn, sq_scale=1.0, eps_tile=None):
                """feed_tile_fn(d) -> bf16 [128,TOK] AP. Returns
                (rstd [128,TOK] f32 broadcast, m_row [1,TOK] bf16)."""
                sum_ps = psp.tile([1, TOK], f32, tag="ps")
                ssq_ps = psp.tile([1, TOK], f32, tag="ps")
                for d in range(DC):
                    xt = feed_tile_fn(d)
                    sq = sqp.tile([128, TOK], bf16, tag="sq")
                    nc.vector.tensor_tensor(sq[:], xt, xt, op=ALU.mult)
                    nc.tensor.matmul(sum_ps[:], ones_col[:], xt,
                                     start=(d == 0), stop=(d == DC - 1))
                    nc.tensor.matmul(ssq_ps[:], ones_col[:], sq[:],
                                     start=(d == 0), stop=(d == DC - 1))
                sum_row = rowp.tile([1, TOK], bf16, tag="row")
                ssq_row = rowp.tile([1, TOK], bf16, tag="row")
                nc.vector.tensor_copy(sum_row[:], sum_ps[:])
                nc.vector.tensor_copy(ssq_row[:], ssq_ps[:])
                mean_ps = psp.tile([128, TOK], f32, tag="ps")
                msq_ps = psp.tile([128, TOK], f32, tag="ps")
                nc.tensor.matmul(mean_ps[:], invD_row[:], sum_row[:],
                                 start=True, stop=True)
                nc.tensor.matmul(msq_ps[:], invD_row[:], ssq_row[:],
                                 start=True, stop=True)
                msq = wfp.tile([128, TOK], f32, tag="wf")
                nc.scalar.activation(msq[:], mean_ps[:], AF.Square)
                var = wfp.tile([128, TOK], f32, tag="wf")
                nc.vector.tensor_tensor(var[:], msq_ps[:], msq[:], op=ALU.subtract)
                std = wfp.tile([128, TOK], f32, tag="wf")
                nc.scalar.activation(std[:], var[:], AF.Sqrt,
                                     bias=eps_col[:] if eps_tile is None
                                     else eps_tile,
                                     scale=sq_scale)
                rstd = rstdp.tile([128, TOK], f32, tag="rstd")
                nc.vector.reciprocal(rstd[:], std[:])
                m_row = rowp.tile([1, TOK], bf16, tag="row")
                nc.vector.tensor_copy(m_row[:], mean_ps[0:1, :])
                return rstd, m_row

            # ================= P1: LN1 + QKV (k, v, then q) =================
            p1s = ExitStack()
            wqp = p1s.enter_context(tc.tile_pool(name="wqp", bufs=6))
            xbp = p1s.enter_context(tc.tile_pool(name="xbp", bufs=1))
            kvoutp = p1s.enter_context(tc.tile_pool(name="kvoutp", bufs=4))

            xb = []
            for d in range(DC):
                t = xbp.tile([128, TOK], bf16, tag=f"xb{d}")
                nc.sync.dma_start(t[:], xbf[128 * d:128 * (d + 1), :])
                xb.append(t)
            x3 = [xbp.tile([128, 2, TOK], f8, tag=f"x3{c}", name=f"x3{c}")
                  for c in range(DC // 2)]
            for d in range(DC):
                nc.vector.tensor_copy(x3[d // 2][:, d % 2, :], xb[d][:])

            # sq_scale=1024 folds the 1/32 fp8 weight scale into rstd1
            rstd1, m1 = ln_stats(lambda d: xb[d][:], sq_scale=1024.0,
                                 eps_tile=eps_col2[:])

            DR = mybir.MatmulPerfMode.DoubleRow
            q_sb = []
            for part in (1, 2, 0):          # k, v, then q
                for j in range(DC):
                    c0 = D * part + 128 * j
                    wqt = wqp.tile([128, DC, 128], f8, tag="wqs")
                    nc.sync.dma_start(wqt[:], wqblk[part, j])
                    ps = psp.tile([128, TOK], f32, tag="ps")
                    for c in range(DC // 2):
                        nc.tensor.matmul(ps[:], wqt[:, 2 * c:2 * c + 2, :],
                                         x3[c][:], perf_mode=DR,
                                         start=(c == 0), stop=False)
                    nc.tensor.matmul(ps[:], ncsq_row[0:1, c0:c0 + 128],
                                     m1[:], start=False, stop=True)
                    if part == 0:
                        t = qp.tile([128, TOK], f8, tag=f"q{j}")
                        nc.vector.tensor_tensor(t[:], ps[:], rstd1[:], op=ALU.mult)
                        q_sb.append(t)
                    else:
                        t = kvoutp.tile([128, TOK], f8, tag="kvout")
                        nc.vector.tensor_tensor(t[:], ps[:], rstd1[:], op=ALU.mult)
                        dst = k_own if part == 1 else v_own
                        nc.sync.dma_start(dst[128 * j:128 * (j + 1), :], t[:])
                if part == 1:
                    nc.gpsimd.collective_compute(
                        "AllGather", ALU.bypass, ins=[k_own.opt()],
                        outs=[k_ag.opt()], replica_groups=RG2)
                elif part == 2:
                    nc.gpsimd.collective_compute(
                        "AllGather", ALU.bypass, ins=[v_own.opt()],
                        outs=[v_ag.opt()], replica_groups=RG2)

            p1s.close()

            # ---- residual-phase pools (live until kernel end) ----
            ress = ExitStack()
            owp = ress.enter_context(tc.tile_pool(name="owp", bufs=1))
            xfp = ress.enter_context(tc.tile_pool(name="xfp", bufs=1))
            aop = ress.enter_context(tc.tile_pool(name="aop", bufs=1))
            resp = ress.enter_context(tc.tile_pool(name="resp", bufs=1))
            resbp = ress.enter_context(tc.tile_pool(name="resbp", bufs=1))

            ow_sb = []
            for oc in range(DC):
                t = owp.tile([128, DC, 128], f8, tag=f"ow{oc}")
                nc.sync.dma_start(t[:], owblk[oc])
                ow_sb.append(t)
            xff = []
            for d in range(DC):
                t = xfp.tile([128, TOK], f32, tag=f"xf{d}")
                nc.sync.dma_start(t[:], xf[128 * d:128 * (d + 1), :])
                xff.append(t)

            # ctxT destination tiles (written via SBUF->SBUF DMA bounce),
            # fp8 pairs for the DoubleRow attn_ow matmul, scaled x64
            ctxT3 = [ctxp.tile([128, 2, TOK], f8, tag=f"ctxT3{c}",
                               name=f"ctxT3{c}") for c in range(DC // 2)]

            # ================= P2: attention, head pairs =================
            p2s = ExitStack()
            kvp = p2s.enter_context(tc.tile_pool(name="kvp", bufs=16))
            vaugp = p2s.enter_context(tc.tile_pool(name="vaugp", bufs=20))
            expp = p2s.enter_context(tc.tile_pool(name="expp", bufs=10))
            rbp = p2s.enter_context(tc.tile_pool(name="rbp", bufs=2))
            cnp = p2s.enter_context(tc.tile_pool(name="cnp", bufs=4))
            vcp = p2s.enter_context(tc.tile_pool(name="vcp", bufs=8))
            cup = p2s.enter_context(tc.tile_pool(name="cup", bufs=4))
            psB = p2s.enter_context(tc.tile_pool(name="psB", bufs=2, space="PSUM"))

            # pre-touch vaug buffers once so the two ones-columns persist
            # across pool rotations (copies below only write the v halves)
            for _ in range(20):
                va = vaugp.tile([128, 130], bf16, tag="vaug")
                nc.gpsimd.memset(va[:, 64:65], 1.0)
                nc.gpsimd.memset(va[:, 129:130], 1.0)

            def load_kv_tiles(p):
                ks, vs = [], []
                for r in range(NR):
                    row0 = D * r + 128 * p
                    kt = kvp.tile([128, TOK], f8, tag="kv")
                    nc.sync.dma_start(kt[:], k_ag[row0:row0 + 128, :])
                    ks.append(kt)
                for r in range(NR):
                    row0 = D * r + 128 * p
                    vt = kvp.tile([128, TOK], f8, tag="kv")
                    nc.sync.dma_start(vt[:], v_ag[row0:row0 + 128, :])
                    vc = vcp.tile([128, TOK], bf16, tag="vc")
                    nc.vector.tensor_copy(vc[:], vt[:])
                    vs.append(vc)
                return ks, vs

            def transpose_v(vs, kc):
                r, jj = kc // (TOK // 128), kc % (TOK // 128)
                tp = psp.tile([128, 128], bf16, tag="ps")
                nc.tensor.transpose(tp[:], vs[r][:, 128 * jj:128 * (jj + 1)],
                                    ident[:])
                va = vaugp.tile([128, 130], bf16, tag="vaug")
                nc.vector.tensor_copy(va[:, 0:64], tp[:, 0:64])
                nc.vector.tensor_copy(va[:, 65:129], tp[:, 64:128])
                return va

            # Per pair, kc groups of 4: scores+exp for group g, transposes
            # for group g, ctx accumulation for group g-1. The lagged ctx
            # keeps the PE busy while ACT works through the exps, so the
            # PE never idles past the HAM re-throttle window.
            GRP = 2
            NG = NKC // GRP
            pending = []

            def flush_normalize():
                while pending:
                    pp, h, cu, rbf = pending.pop(0)
                    rbps = psp.tile([64, TOK], f32, tag="ps")
                    nc.tensor.matmul(rbps[:], ones_all[64:65, :],
                                     rbf[64:65, :], start=True, stop=True)
                    rb_sb = rbp.tile([64, TOK], f32, tag="rbsb")
                    nc.vector.tensor_copy(rb_sb[:], rbps[:])
                    cn = cnp.tile([64, TOK], f8, tag="cn")
                    nc.vector.scalar_tensor_tensor(cn[:], cu[0:64, :], 64.0,
                                                   rb_sb[:], op0=ALU.mult,
                                                   op1=ALU.mult)
                    nc.sync.dma_start(
                        ctxT3[pp // 2][64 * h:64 * h + 64, pp % 2, :], cn[:])

            kv_tiles = load_kv_tiles(0)
            for p in range(H // 2):
                ks, vs = kv_tiles
                if p + 1 < H // 2:
                    kv_tiles = load_kv_tiles(p + 1)

                cps = [psp.tile([65, TOK], f32, tag="ps", name=f"cps{p}_{hh}")
                       for hh in range(2)]
                exps = [None] * NKC
                vaug = [None] * NKC

                def ctx_group(g):
                    for h in range(2):
                        for kc in range(GRP * g, GRP * (g + 1)):
                            nc.tensor.matmul(
                                cps[h][:], vaug[kc][:, 65 * h:65 * h + 65],
                                exps[kc][:, TOK * h:TOK * (h + 1)],
                                start=(kc == 0), stop=(kc == NKC - 1))

                for g in range(NG):
                    if g == 2 and pending:
                        flush_normalize()
                    for kc in range(GRP * g, GRP * (g + 1)):
                        r, jj = kc // (TOK // 128), kc % (TOK // 128)
                        csl = slice(128 * jj, 128 * (jj + 1))
                        sps = psB.tile([128, 2 * TOK], f32, tag="ps2")
                        nc.tensor.matmul(sps[:, 0:TOK], ks[r][0:64, csl],
                                         q_sb[p][0:64, :],
                                         start=True, stop=True)
                        nc.tensor.matmul(sps[:, TOK:2 * TOK], ks[r][64:128, csl],
                                         q_sb[p][64:128, :],
                                         start=True, stop=True)
                        e = expp.tile([128, 2 * TOK], bf16, tag="exp")
                        nc.scalar.activation(e[:], sps[:], AF.Exp,
                                             scale=1.0 / np.sqrt(HD))
                        exps[kc] = e
                        vaug[kc] = transpose_v(vs, kc)
                    if g > 0:
                        ctx_group(g - 1)
                ctx_group(NG - 1)

                # drain ctx+denominator to SBUF, start the reciprocal; the
                # PE-side broadcast + normalize is deferred into the next
                # pair so the slow DVE reciprocal never stalls the PE queue
                for h in range(2):
                    cu = cup.tile([65, TOK], f32, tag="cu", name=f"cu{p}_{h}")
                    nc.vector.tensor_copy(cu[:], cps[h][:])
                    lnd = rbp.tile([128, TOK], f32, tag="rr")
                    nc.scalar.activation(lnd[64:65, :], cu[64:65, :], AF.Ln)
                    rbf = rbp.tile([128, TOK], bf16, tag="rbf")
                    nc.scalar.activation(rbf[64:65, :], lnd[64:65, :], AF.Exp,
                                         scale=-1.0)
                    pending.append((p, h, cu, rbf))

            flush_normalize()
            p2s.close()

            # ================= P3: attn_ow + residual + LN2 =================
            # ao (x64) and resid are packed into fp8 [128,2,TOK] pair tiles
            # for the DoubleRow MLP matmuls
            ao3 = [aop.tile([128, 2, TOK], f8, tag=f"ao3{c}", name=f"ao3{c}")
                   for c in range(DC // 2)]
            res3 = [resbp.tile([128, 2, TOK], f8, tag=f"res3{c}", name=f"res3{c}")
                    for c in range(DC // 2)]
            res_f, res_b = [], []
            for oc in range(DC):
                pps = psp.tile([128, TOK], f32, tag="ps")
                for c in range(DC // 2):
                    nc.tensor.matmul(pps[:], ow_sb[oc][:, 2 * c:2 * c + 2, :],
                                     ctxT3[c][:], perf_mode=DR,
                                     start=(c == 0), stop=(c == DC // 2 - 1))
                # psum carries 64*32 = 2048x scale from ctx and ow fp8
                nc.vector.tensor_scalar(ao3[oc // 2][:, oc % 2, :], pps[:],
                                        1.0 / 32.0, None, op0=ALU.mult)
                rf = resp.tile([128, TOK], f32, tag=f"res{oc}")
                nc.vector.scalar_tensor_tensor(rf[:], pps[:], 1.0 / 2048.0,
                                               xff[oc][:], op0=ALU.mult,
                                               op1=ALU.add)
                res_f.append(rf)
                rb = resbp.tile([128, TOK], bf16, tag=f"resb{oc}")
                nc.vector.tensor_copy(rb[:], rf[:])
                res_b.append(rb)
                nc.vector.tensor_copy(res3[oc // 2][:, oc % 2, :], rf[:])

            # sq_scale=1024 folds the 1/32 fp8 weight scale into rstd2
            rstd2, m2 = ln_stats(lambda d: res_b[d][:], sq_scale=1024.0,
                                 eps_tile=eps_col2[:])

            # ================= P4: MLP =================
            p4s = ExitStack()
            intp = p4s.enter_context(tc.tile_pool(name="intp", bufs=1))
            h1p = p4s.enter_context(tc.tile_pool(name="h1p", bufs=3))
            w1sp = p4s.enter_context(tc.tile_pool(name="w1sp", bufs=3))
            w2sp = p4s.enter_context(tc.tile_pool(name="w2sp", bufs=3))
            outwsp = p4s.enter_context(tc.tile_pool(name="outwsp", bufs=2))
            ofp = p4s.enter_context(tc.tile_pool(name="ofp", bufs=3))

            inter3 = [intp.tile([128, 2, TOK], f8, tag=f"it3{i}", name=f"it3{i}")
                      for i in range(IC // 2)]
            for ic in range(IC):
                w1t = w1sp.tile([128, DC, 128], f8, tag="w1s")
                nc.sync.dma_start(w1t[:], w1blk[ic])
                h1ps = psp.tile([128, TOK], f32, tag="ps")
                for c in range(DC // 2):
                    nc.tensor.matmul(h1ps[:], w1t[:, 2 * c:2 * c + 2, :],
                                     res3[c][:], perf_mode=DR,
                                     start=(c == 0), stop=False)
                nc.tensor.matmul(h1ps[:], ncs1_row[0:1, 128 * ic:128 * (ic + 1)],
                                 m2[:], start=False, stop=True)
                gi = wfp.tile([128, TOK], f32, tag="wf")
                nc.vector.tensor_tensor(gi[:], h1ps[:], rstd2[:], op=ALU.mult)
                h1 = h1p.tile([128, TOK], bf16, tag="h1")
                nc.scalar.activation(h1[:], gi[:], AF.Gelu)

                w2t = w2sp.tile([128, DC, 128], f8, tag="w2s")
                nc.sync.dma_start(w2t[:], w2blk[ic])
                h2ps = psp.tile([128, TOK], f32, tag="ps")
                for c in range(DC // 2):
                    nc.tensor.matmul(h2ps[:], w2t[:, 2 * c:2 * c + 2, :],
                                     ao3[c][:], perf_mode=DR,
                                     start=(c == 0), stop=(c == DC // 2 - 1))
                nc.vector.tensor_tensor(inter3[ic // 2][:, ic % 2, :],
                                        h2ps[:], h1[:], op=ALU.mult)

            for oc in range(DC):
                owt = outwsp.tile([128, IC, 128], f8, tag="outws")
                nc.sync.dma_start(owt[:], outwblk[oc])
                ops = psp.tile([128, TOK], f32, tag="ps")
                for i in range(IC // 2):
                    nc.tensor.matmul(ops[:], owt[:, 2 * i:2 * i + 2, :],
                                     inter3[i][:], perf_mode=DR,
                                     start=(i == 0), stop=(i == IC // 2 - 1))
                of = ofp.tile([128, TOK], f32, tag="of")
                # out = mlp/65536 + resid  (2048 * 32 fp8 scales undone)
                nc.vector.scalar_tensor_tensor(of[:], ops[:], 1.0 / 65536.0,
                                               res_f[oc][:], op0=ALU.mult,
                                               op1=ALU.add)
                nc.sync.dma_start(outT[128 * oc:128 * (oc + 1), :], of[:])

            p4s.close()
            ress.close()

    nc.compile()
    return nc


_NC_CACHE = {}


def kernel(**inputs):
    x = np.asarray(inputs["x"], np.float32)
    norm_w = np.asarray(inputs["norm_w"], np.float32)
    norm_b = np.asarray(inputs["norm_b"], np.float32)
    qkvw = np.asarray(inputs["attn_qkvw"], np.float32)
    qkvb = np.asarray(inputs["attn_qkvb"], np.float32)
    attn_ow = np.asarray(inputs["attn_ow"], np.float32)
    attn_ob = np.asarray(inputs["attn_ob"], np.float32)
    attn_nw = np.asarray(inputs["attn_nw"], np.float32)
    attn_nb = np.asarray(inputs["attn_nb"], np.float32)
    inter_w = np.asarray(inputs["inter_w"], np.float32)
    inter_b = np.asarray(inputs["inter_b"], np.float32)
    inter_w1 = np.asarray(inputs["inter_w1"], np.float32)
    output_w = np.asarray(inputs["output_w"], np.float32)
    output_b = np.asarray(inputs["output_b"], np.float32)

    X = x.reshape(T, D)
    XT = np.ascontiguousarray(X.T)          # [D, T]

    # ---- LN folds (host) ----
    wqkv_f = norm_w[:, None] * qkvw          # [D, 3D]
    bqkv_f = qkvb + norm_b @ qkvw

    w1_f = attn_nw[:, None] * inter_w        # [D, I]
    b1_f = inter_b + attn_nb @ inter_w

    assert not np.any(bqkv_f) and not np.any(attn_ob) and not np.any(b1_f) \
        and not np.any(output_b), "nonzero biases not wired in this build"

    wq_s = 32.0 * wqkv_f
    wqblk = _f8(wq_s.reshape(DC, 128, 3, DC, 128).transpose(2, 3, 1, 0, 4))
    ncsq = _bf16(-wq_s.sum(0, keepdims=True))
    owblk = _f8((32.0 * attn_ow).reshape(DC, 128, DC, 128).transpose(2, 1, 0, 3))
    w1_s = 32.0 * w1_f
    w1blk = _f8(w1_s.reshape(DC, 128, IC, 128).transpose(2, 1, 0, 3))
    ncs1 = _bf16(-w1_s.sum(0, keepdims=True))
    w2blk = _f8((32.0 * inter_w1).reshape(DC, 128, IC, 128).transpose(2, 1, 0, 3))
    outwblk = _f8((32.0 * output_w).reshape(IC, 128, DC, 128)
                  .transpose(2, 1, 0, 3))

    if "nc" not in _NC_CACHE:
        _NC_CACHE["nc"] = _build()
    nc = _NC_CACHE["nc"]

    in_maps = []
    for c in range(NC):
        tsl = slice(TOK * c, TOK * (c + 1))
        xs = np.ascontiguousarray(XT[:, tsl])
        in_maps.append({
            "xbf": _bf16(xs),
            "xf": xs,
            "wqblk": wqblk,
            "ncsq": ncsq,
            "owblk": owblk,
            "w1blk": w1blk,
            "ncs1": ncs1,
            "w2blk": w2blk,
            "outwblk": outwblk,
        })

    global _LAST_IN_MAPS
    _LAST_IN_MAPS = in_maps
    res = run_bass_kernel_spmd(nc, in_maps, list(range(NC)))
    outT = np.concatenate([res.results[c]["outT"] for c in range(NC)], axis=1)
    return np.ascontiguousarray(outT.T).reshape(B, S, D).astype(np.float32)


if __name__ == "__main__":
    pass

